# revision 35
# baseline (speedup 1.0000x reference)
"""Trainium2 Bass kernel for AffinityNeuralNetworkMONN (gnn_message_passing), v8.

v2 (503us) -> v8 (391us): software-pipelined emission. The v2 HW profile showed
PE busy 341us with 163us of idle gaps: the per-graph chains serialize
PE -> Act -> DVE -> PE within each graph, and every matmul pays a serialized
~107ns LDWEIGHTS (walrus rejects ldw-opt for this kernel's f32r matmuls).

v3 changes:
- The per-iteration j-loop is split into an m-independent "front"
  (ppre/hp0/c2p matmuls + acts + qraw = c2p*hp0) and an m-dependent "back"
  (score matmuls, hcw, inner matmuls, scj). Fronts run 2 graphs ahead of
  backs so PE always has independent queued work while Act/DVE drain.
- The tanh(m@W_mp1) scale is folded into the score matmul's stationary
  (wbas: per-graph basis columns holding wp_w) instead of into the qpt DVE
  op, making fronts fully m-independent; the first two fronts of iteration
  i+1 are emitted inside iteration i's tail to cover the softmax/GRU chain.
- PSUM repacked: 5x [128,512] rotating half-tiles for every big matmul
  (ppre/hp0/c2p/inner/phaseA; the scatter-sum accumulates per half + a tiny
  add), 1x [128,1024] alternating scores/softmax-scratch, 1x [128,512]
  transpose bank. Score halves live at partitions 0:8 of one 2-bank tile,
  so exp/esum stay partition-aligned.
- esum via exp-act accum_out instead of a separate DVE reduce.
- Phase A staggered the same way (peT/ppT matmuls of graph j+1 emitted
  before the transposes/pair matmuls of graph j).
All f32r (bf16 fails: c-softmax |scores|~100 amplifies any perturbation;
measured 5-9e-2 rel err for bf16 variants vs the 2e-2 gate).
Sharding: data-parallel over B=128 graphs -> 8 NeuronCores x 16 graphs.
"""
import sys
for p in ("/opt/trn_rl_repo", "/root/.axon_site/_ro/trn_rl_repo"):
    if p not in sys.path:
        sys.path.insert(0, p)

import numpy as np
from contextlib import ExitStack

import concourse.bass as bass
import concourse.tile as tile
from concourse import mybir, masks
from concourse.bass_utils import run_bass_kernel_spmd

F32 = mybir.dt.float32
F32R = mybir.dt.float32r
BF16 = mybir.dt.bfloat16
A = mybir.ActivationFunctionType
OP = mybir.AluOpType
AX = mybir.AxisListType

NCORES = 8
B, NC, NP, H, D = 128, 64, 1024, 128, 3
G = B // NCORES            # graphs per core = 16
WV = 8                     # graphs per wave
NW = G // WV               # waves = 2
NCH = NP // 128            # 8 p-chunks per graph

# wpack column layout (all f32)
W_PC, W_CAFF, W_PP, W_PAFF = 0, 128, 256, 384
def W_P2C(i): return 512 + i * 128
def W_HP0(i): return 896 + i * 128
def W_HC0(i): return 1280 + i * 128
def W_C2P(i): return 1664 + i * 128
def W_MCP(i): return 2048 + i * 256
W_IH, W_HH, W_SAFF = 2816, 3200, 3584
def W_HC1(i): return 3712 + i
def W_HP1(i): return 3715 + i
W_ONES = 3718
WCOLS = 3719

# bpack cols (per-partition biases for T-layout activations)
B_PP, B_PAFF, B_PC, B_CAFF = 0, 1, 2, 3
def B_P2C(i): return 4 + i
def B_HP0(i): return 7 + i
def B_HC0(i): return 10 + i
def B_C2P(i): return 13 + i
def B_HP1(i): return 16 + i   # scalar b_hp1 replicated across partitions
B_ONES = 19
BCOLS = 20

# bg (per-wave, [WV, x]): per-graph-row biases along the free dim
BG_SAFF = 0
def BG_MCP(i): return 128 + i * 256
BG_IH, BG_HH = 896, 1280
BGCOLS = 1664

# sm1 shared-bank column map ([128, 1024] fp32 = 2 PSUM banks)
PF0 = 0          # pf col per graph            (0:8)
CF0 = 8          # cf col per graph            (8:16)
BC0 = 16         # 1/esum broadcast            (16:24)
CS0 = 24         # cesum (preamble)            (24:40)
FIN0 = 40        # final output col
GI0 = 128        # GRU gi matmul region        (128:512), rows 0:WV

LPERM = [0, 1, 2, 3, 4, 5, 6, 7]
GPERM = [w * WV + LPERM[j] for w in range(G // WV) for j in range(WV)]

_CACHE = {}
TRACE = False
LAST_EXEC_NS = None
LAST_RESULT = None


def _split_waits(nc, keep=1):
    """walrus allows very few attached sync-waits per instruction (1 for the
    f32 self-loading matmul struct). Hoist excess waits into standalone
    EventSemaphore instructions right before the over-subscribed one."""
    for fn in nc.m.functions:
        for blk in fn.blocks:
            out = []
            for ins in blk.instructions:
                si = ins.sync_info
                if si is not None and si.on_wait and len(si.on_wait) > keep:
                    waits = list(si.on_wait)
                    for jj, w in enumerate(waits[:-keep]):
                        ev = mybir.InstNoOp(
                            name=f"{ins.name}-wsplit{jj}",
                            sync_info=mybir.SyncInfo(on_wait=[w], on_update=[]),
                            bass_nofuse=True)
                        ev.engine = ins.engine
                        out.append(ev)
                    si.on_wait = waits[-keep:]
                    ins.sync_info = si
                out.append(ins)
            blk.instructions = out


def _build(b_out_val: float, b_hp1: list, split: bool = True):
    nc = bass.Bass()
    protT_d = nc.dram_tensor("protT", [H, G * NP], F32, kind="ExternalInput")
    compT_d = nc.dram_tensor("compT", [H, G * NC], F32, kind="ExternalInput")
    gompT_d = nc.dram_tensor("gompT", [H, G], F32, kind="ExternalInput")
    wpack_d = nc.dram_tensor("wpack", [H, WCOLS], F32, kind="ExternalInput")
    bpack_d = nc.dram_tensor("bpack", [H, BCOLS], F32, kind="ExternalInput")
    bg_d = nc.dram_tensor("bgw", [WV, BGCOLS], F32, kind="ExternalInput")
    w2t_d = nc.dram_tensor("w2t", [H, 2 * H], F32, kind="ExternalInput")
    out_d = nc.dram_tensor("out", [G, 1], F32, kind="ExternalOutput")

    with tile.TileContext(nc) as tc, ExitStack() as ctx:
        gl = ctx.enter_context(tc.tile_pool(name="globals", bufs=1))
        pw = ctx.enter_context(tc.tile_pool(name="perwave", bufs=1))
        st = ctx.enter_context(tc.tile_pool(name="stream", bufs=2))
        sm = ctx.enter_context(tc.tile_pool(name="small", bufs=1))
        ps_bp = ctx.enter_context(tc.tile_pool(name="psB", bufs=5, space="PSUM"))
        ps_sp = ctx.enter_context(tc.tile_pool(name="psS", bufs=1, space="PSUM"))
        ps_tp = ctx.enter_context(tc.tile_pool(name="psT", bufs=1, space="PSUM"))

        def psb():
            return ps_bp.tile([H, 512], F32, name="psb", tag="half")

        def pssm():
            return ps_sp.tile([H, 1024], F32, name="pssm", tag="small")

        def pst():
            return ps_tp.tile([H, 512], F32, name="pst", tag="pst")

        # ---------- globals ----------
        wp = gl.tile([H, WCOLS], F32R, name="wp", tag="wp")
        bp = gl.tile([H, BCOLS], F32, name="bp", tag="bp")
        bg = gl.tile([WV, BGCOLS], F32, name="bg", tag="bg")
        w2t = gl.tile([H, 2 * H], F32, name="w2t", tag="w2t")
        compT = st.tile([H, G * NC], F32R, name="compT", tag="qraw", bufs=3)
        gompT = gl.tile([H, G], F32R, name="gompT", tag="gompT")
        nc.sync.dma_start(out=wp[:, 0:512],
                          in_=wpack_d[:, 0:512].bitcast(F32R))
        nc.sync.dma_start(out=compT[:], in_=compT_d[:].bitcast(F32R))
        nc.sync.dma_start(out=bp[:], in_=bpack_d[:])

        ident = gl.tile([H, H], F32, name="ident", tag="ident")
        masks.make_identity(nc, ident[:])
        onesr = gl.tile([1, H], F32, name="onesr", tag="onesr")
        nc.vector.memset(onesr[:], 1.0)
        # wbas: 8 blocks of [H, 8]; block j col j = tanh(m@W_mp1)*w_hp1 per
        # iteration (the rest stays 0 from this one-time clear).
        wbas = gl.tile([H, WV * WV], F32R, name="wbas", tag="wbas")
        nc.vector.memset(wbas[:].bitcast(F32), 0.0)

        # pcT = lrelu(W_pc^T comp), ceT = lrelu(W_caff^T comp)  [h, (g c)]
        pcT = gl.tile([H, G * NC], F32R, name="pcT", tag="pcT")
        ceT = gl.tile([H, G * NC], F32R, name="ceT", tag="ceT")
        for dst, wcol, bcol in ((pcT, W_PC, B_PC), (ceT, W_CAFF, B_CAFF)):
            for half in range(2):
                ps_h = psb()
                nc.tensor.matmul(ps_h[:], wp[:, wcol:wcol + H],
                                 compT[:, half * 512:(half + 1) * 512],
                                 start=True, stop=True)
                nc.scalar.activation(dst[:, half * 512:(half + 1) * 512],
                                     ps_h[:], A.Prelu,
                                     bias=bp[:, bcol:bcol + 1], alpha=0.1)

        # ce_nm [64, G, H]  (c on partitions 0:64, one slot per graph)
        ce_nm = gl.tile([64, G, H], F32R, name="ce_nm", tag="ce_nm")
        for q4 in range(4):
            psq = pst()
            for k in range(4):
                g = q4 * 4 + k
                nc.tensor.transpose(psq[0:64, k * H:(k + 1) * H],
                                    ceT[:, g * NC:(g + 1) * NC].bitcast(F32),
                                    ident[:])
            nc.vector.tensor_copy(ce_nm[:, q4 * 4:(q4 + 1) * 4, :],
                                  psq[0:64, :].rearrange("c (k h) -> c k h", k=4))

        # cesum[h, q] = sum_c ce[c, h] of graph GPERM[q]
        ps_cs = pssm()
        for q in range(G):
            g = GPERM[q]
            nc.tensor.matmul(ps_cs[:, CS0 + q:CS0 + q + 1],
                             ce_nm[:, g, :].bitcast(F32),
                             bp[0:64, B_ONES:B_ONES + 1],
                             start=True, stop=True)
        cesum = gl.tile([H, G], F32, name="cesum", tag="cesum")
        nc.vector.tensor_copy(cesum[:], ps_cs[:, CS0:CS0 + G])

        # peacc halves: act accum per 512-col half -> [H, 2G]
        peacc = gl.tile([H, 2 * G], F32, name="peacc", tag="peacc")
        partials = gl.tile([H, G], F32R, name="partials", tag="partials")

        xcf_w, pfn_w, sf_ww, acf_w = [], [], [], []

        # ---------- per-wave processing ----------
        for w in range(NW):
            gs = w * WV

            peT = [pw.tile([H, NP], F32R, name=f"peT{j}", tag=f"peT{j}")
                   for j in range(WV)]
            pairs = [pw.tile([64, NP], F32R, name=f"pairs{j}", tag=f"pairs{j}")
                     for j in range(WV)]
            penm = [pw.tile([H, NCH, H], BF16, name=f"penm{j}", tag=f"penm{j}")
                    for j in range(WV)]
            ew = pw.tile([H, NCH, WV], BF16, name="ew", tag="ew")
            cpre_all = {i: pw.tile([64, WV, H], F32R, name=f"cpre_nm{i}",
                                   tag=f"cpre_nm{i}") for i in range(D)}
            hc0_all = {i: pw.tile([H, WV * NC], F32, name=f"hc0T{i}",
                                  tag=f"hc0T{i}") for i in range(D)}

            # ----- phase A (staggered: A1(j+1) before A2(j)) -----
            ppT_t = {}

            def phaseA1(j):
                g = gs + LPERM[j]
                q = gs + j
                protT = st.tile([H, NP], F32R, name="protT", tag="protT",
                                bufs=3)
                nc.sync.dma_start(out=protT[:],
                                  in_=protT_d[:, g * NP:(g + 1) * NP].bitcast(F32R))
                # peT = lrelu(W_paff^T prot + b); halves; accum -> peacc
                for half in range(2):
                    ps_h = psb()
                    nc.tensor.matmul(ps_h[:], wp[:, W_PAFF:W_PAFF + H],
                                     protT[:, half * 512:(half + 1) * 512],
                                     start=True, stop=True)
                    nc.scalar.activation(
                        peT[j][:, half * 512:(half + 1) * 512], ps_h[:],
                        A.Prelu, bias=bp[:, B_PAFF:B_PAFF + 1], alpha=0.1,
                        accum_out=peacc[:, 2 * q + half:2 * q + half + 1])
                # ppT = lrelu(W_pp^T prot + b)
                ppT = st.tile([H, NP], F32R, name="ppT", tag="qraw", bufs=3)
                for half in range(2):
                    ps_h = psb()
                    nc.tensor.matmul(ps_h[:], wp[:, W_PP:W_PP + H],
                                     protT[:, half * 512:(half + 1) * 512],
                                     start=True, stop=True)
                    nc.scalar.activation(ppT[:, half * 512:(half + 1) * 512],
                                         ps_h[:], A.Prelu,
                                         bias=bp[:, B_PP:B_PP + 1], alpha=0.1)
                ppT_t[j] = ppT

            def phaseA2(j):
                g = gs + LPERM[j]
                ppT = ppT_t.pop(j)
                # pe_nm chunks (bf16) via transposes of peT
                for half in range(2):
                    psq = pst()
                    for k in range(4):
                        ch = half * 4 + k
                        nc.tensor.transpose(psq[:, k * H:(k + 1) * H],
                                            peT[j][:, ch * H:(ch + 1) * H].bitcast(F32),
                                            ident[:])
                    nc.vector.tensor_copy(
                        penm[j][:, half * 4:(half + 1) * 4, :],
                        psq[:].rearrange("h (k c) -> h k c", k=4))
                # pair = sigmoid(pc^T @ ppT)  [64, NP]
                pcsec = pcT[:, g * NC:(g + 1) * NC]
                for half in range(2):
                    ps_h = psb()
                    nc.tensor.matmul(ps_h[0:64, :], pcsec,
                                     ppT[:, half * 512:(half + 1) * 512],
                                     start=True, stop=True)
                    nc.scalar.activation(ps_h[0:64, :], ps_h[0:64, :],
                                         A.Tanh, scale=0.5)
                    nc.vector.tensor_scalar(
                        pairs[j][:, half * 512:(half + 1) * 512],
                        ps_h[0:64, :], 0.5, 0.5, OP.mult, OP.add)

            phaseA1(0)
            for j in range(1, WV):
                phaseA1(j)
                if w == 0 and j == 2:
                    # deferred bulk loads: queue behind the first protein
                    # tiles so phase A starts ~4.6us earlier
                    nc.sync.dma_start(out=bg[:], in_=bg_d[:])
                    nc.sync.dma_start(out=gompT[:],
                                      in_=gompT_d[:].bitcast(F32R))
                elif w == 0 and j == 4:
                    nc.sync.dma_start(out=wp[:, 512:WCOLS],
                                      in_=wpack_d[:, 512:WCOLS].bitcast(F32R))
                    nc.sync.dma_start(out=w2t[:], in_=w2t_d[:])
                phaseA2(j - 1)
            phaseA2(WV - 1)

            # m0 = (cesum * (peacc_even + peacc_odd)) / (NC*NP)
            pesum = sm.tile([H, WV], F32, name="pesum", tag="pesum", bufs=1)
            nc.vector.tensor_add(pesum[:],
                                 peacc[:, 2 * gs:2 * (gs + WV):2],
                                 peacc[:, 2 * gs + 1:2 * (gs + WV):2])
            mT = sm.tile([H, WV], F32R, name="mT", tag="mT")
            nc.vector.scalar_tensor_tensor(mT[:], cesum[:, gs:gs + WV],
                                           1.0 / (NC * NP), pesum[:],
                                           OP.mult, OP.mult)
            ps_m0 = pst()
            nc.tensor.transpose(ps_m0[0:WV, 0:H], mT[:].bitcast(F32), ident[:])
            m_nm = sm.tile([WV, H], F32, name="m_nm", tag="m_nm", bufs=1)
            nc.vector.tensor_copy(m_nm[:], ps_m0[0:WV, 0:H])

            # sf = lrelu(gomp @ W_saff + b_saff)   [WV, H]
            ps_sf = pst()
            nc.tensor.matmul(ps_sf[0:WV, 0:H], gompT[:, gs:gs + WV],
                             wp[:, W_SAFF:W_SAFF + H], start=True, stop=True)
            sf_pre = sm.tile([WV, H], F32, name="sf_pre", tag="sf_pre", bufs=1)
            nc.vector.tensor_add(sf_pre[:], ps_sf[0:WV, 0:H],
                                 bg[:, BG_SAFF:BG_SAFF + H])
            sf_w = sm.tile([WV, H], F32R, name="sf_w", tag=f"sf_w{w}", bufs=1)
            nc.scalar.activation(sf_w[:], sf_pre[:], A.Prelu, alpha=0.1)
            sf_ww.append(sf_w)

            # per-iter c-side tensors (m-independent, all iterations upfront)
            ceW = ceT[:, gs * NC:(gs + WV) * NC]
            for i in range(D):
                ps_cpre = psb()
                nc.tensor.matmul(ps_cpre[:], wp[:, W_C2P(i):W_C2P(i) + H],
                                 ceW, start=True, stop=True)
                cpreT = st.tile([H, 512], F32, name="cpreT", tag="cpreTs",
                                bufs=2)
                nc.scalar.activation(cpreT[:], ps_cpre[:], A.Tanh,
                                     bias=bp[:, B_C2P(i):B_C2P(i) + 1])
                cpre_nm = cpre_all[i]
                for half in range(2):
                    psq = pst()
                    for k in range(4):
                        lg = half * 4 + k
                        nc.tensor.transpose(psq[0:64, k * H:(k + 1) * H],
                                            cpreT[:, lg * NC:(lg + 1) * NC],
                                            ident[:])
                    nc.vector.tensor_copy(
                        cpre_nm[:, half * 4:(half + 1) * 4, :],
                        psq[0:64, :].rearrange("c (k h) -> c k h", k=4))
                ps_h0 = psb()
                nc.tensor.matmul(ps_h0[:], wp[:, W_HC0(i):W_HC0(i) + H],
                                 ceW, start=True, stop=True)
                nc.scalar.activation(hc0_all[i][:], ps_h0[:], A.Tanh,
                                     bias=bp[:, B_HC0(i):B_HC0(i) + 1])

            xcf = sm.tile([WV, H], F32, name="xcf", tag=f"xcf{w}", bufs=1)
            pfn_nm = sm.tile([WV, H], F32R, name="pfn_nm", tag=f"pfn_nm{w}",
                             bufs=1)
            xcf_w.append(xcf)
            pfn_w.append(pfn_nm)

            # ----- iterations (software-pipelined) -----
            STATE = {}
            FRONTS = {}

            def mcp_chain(i, mT_cur):
                ps_mcp = pst()
                nc.tensor.matmul(ps_mcp[0:WV, 0:256], mT_cur[:],
                                 wp[:, W_MCP(i):W_MCP(i) + 256],
                                 start=True, stop=True)
                mcp_pre = sm.tile([WV, 256], F32, name="mcp_pre", tag="t256",
                                  bufs=1)
                nc.vector.tensor_add(mcp_pre[:], ps_mcp[0:WV, 0:256],
                                     bg[:, BG_MCP(i):BG_MCP(i) + 256])
                mcp = sm.tile([WV, 256], F32, name="mcp", tag="rz_t", bufs=1)
                nc.scalar.activation(mcp[:], mcp_pre[:], A.Tanh)
                ps_wt = pst()
                nc.tensor.transpose(ps_wt[0:H, 0:WV], mcp[:, 0:H],
                                    ident[0:WV, 0:WV])
                nc.tensor.transpose(ps_wt[0:H, WV:2 * WV], mcp[:, H:256],
                                    ident[0:WV, 0:WV])
                wc_w = sm.tile([H, WV], F32R, name="wc_w", tag="wc_w")
                nc.vector.tensor_scalar(wc_w[:], ps_wt[0:H, 0:WV],
                                        wp[:, W_HC1(i):W_HC1(i) + 1].bitcast(F32),
                                        None, OP.mult)
                # wbas diagonal: block j, col j  (stride WV+1)
                nc.vector.tensor_scalar(
                    wbas[:, 0:WV * WV:WV + 1],
                    ps_wt[0:H, WV:2 * WV],
                    wp[:, W_HP1(i):W_HP1(i) + 1].bitcast(F32), None, OP.mult)
                ps_sc = pssm()
                sc_sb = sm.tile([64, WV], F32, name="sc_sb", tag="sc_sb",
                                bufs=1)
                STATE[i] = {"sc": ps_sc, "sc_sb": sc_sb, "wc": wc_w}

            def front(i, j):
                l = LPERM[j]
                ppreT = st.tile([H, NP], F32R, name="ppreT", tag="ppreT",
                                bufs=3)
                for half in range(2):
                    ps_h = psb()
                    nc.tensor.matmul(ps_h[:], wp[:, W_P2C(i):W_P2C(i) + H],
                                     peT[j][:, half * 512:(half + 1) * 512],
                                     start=True, stop=True)
                    nc.scalar.activation(
                        ppreT[:, half * 512:(half + 1) * 512], ps_h[:],
                        A.Tanh, bias=bp[:, B_P2C(i):B_P2C(i) + 1])
                hp0T = st.tile([H, NP], F32, name="hp0T", tag="hp0T", bufs=2)
                for half in range(2):
                    ps_h = psb()
                    nc.tensor.matmul(ps_h[:], wp[:, W_HP0(i):W_HP0(i) + H],
                                     peT[j][:, half * 512:(half + 1) * 512],
                                     start=True, stop=True)
                    nc.scalar.activation(
                        hp0T[:, half * 512:(half + 1) * 512], ps_h[:],
                        A.Tanh, bias=bp[:, B_HP0(i):B_HP0(i) + 1])
                qraw = st.tile([H, NP], F32R, name="qraw", tag="qraw", bufs=3)
                lhs_cp = cpre_all[i][:, l, :]
                for half in range(2):
                    ps_h = psb()
                    nc.tensor.matmul(ps_h[:], lhs_cp,
                                     pairs[j][:, half * 512:(half + 1) * 512],
                                     start=True, stop=True)
                    nc.vector.scalar_tensor_tensor(
                        qraw[:, half * 512:(half + 1) * 512], ps_h[:], 1.0,
                        hp0T[:, half * 512:(half + 1) * 512],
                        OP.mult, OP.mult)
                FRONTS[(i, j)] = (ppreT, qraw)

            def back(i, j):
                l = LPERM[j]
                stt = STATE[i]
                ppreT, qraw = FRONTS.pop((i, j))
                ps_sc = stt["sc"]
                bas = wbas[:, WV * j:WV * j + WV]
                nc.tensor.matmul(ps_sc[0:WV, 0:512], bas, qraw[:, 0:512],
                                 start=(j == 0), stop=(j == WV - 1))
                nc.tensor.matmul(ps_sc[0:WV, 512:1024], bas,
                                 qraw[:, 512:1024],
                                 start=(j == 0), stop=(j == WV - 1))
                hcw = st.tile([H, 64], F32R, name="hcw", tag="hcw", bufs=2)
                nc.vector.tensor_scalar(hcw[:],
                                        hc0_all[i][:, l * NC:(l + 1) * NC],
                                        stt["wc"][:, j:j + 1].bitcast(F32),
                                        None, OP.mult)
                scp = sm.tile([64, 2], F32, name="scp", tag="scp", bufs=2)
                scjd = st.tile([H, NP], F32, name="scjd", tag="hp0T", bufs=2)
                for half in range(2):
                    ps_in = psb()
                    nc.tensor.matmul(ps_in[0:64, :], hcw[:],
                                     ppreT[:, half * 512:(half + 1) * 512],
                                     start=True, stop=True)
                    nc.vector.scalar_tensor_tensor(
                        scjd[0:64, half * 512:(half + 1) * 512],
                        pairs[j][:, half * 512:(half + 1) * 512].bitcast(F32),
                        1.0, ps_in[0:64, :], OP.mult, OP.mult,
                        accum_out=scp[:, half:half + 1])
                nc.vector.tensor_add(stt["sc_sb"][:, j:j + 1],
                                     scp[:, 0:1], scp[:, 1:2])

            mcp_chain(0, mT)
            front(0, 0)
            front(0, 1)

            for i in range(D):
                stt = STATE[i]
                for j in range(WV):
                    if j + 2 < WV:
                        front(i, j + 2)
                    elif j + 2 == WV and i + 1 < D:
                        front(i + 1, 0)
                    back(i, j)
                ps_sc = stt["sc"]

                # --- p softmax: exp rows (+esum via act accum) ---
                e_sb = st.tile([WV, NP], F32, name="e_sb", tag="hp0T", bufs=2)
                es_p = sm.tile([WV, 2], F32, name="es_p", tag="es_p", bufs=1)
                nc.scalar.activation(e_sb[:, 0:512], ps_sc[0:WV, 0:512],
                                     A.Exp, bias=bp[0:WV, B_HP1(i):B_HP1(i) + 1],
                                     accum_out=es_p[:, 0:1])
                nc.scalar.activation(e_sb[:, 512:1024], ps_sc[0:WV, 512:1024],
                                     A.Exp, bias=bp[0:WV, B_HP1(i):B_HP1(i) + 1],
                                     accum_out=es_p[:, 1:2])
                esum = sm.tile([WV, 1], F32, name="esum", tag="esum")
                nc.vector.tensor_add(esum[:], es_p[:, 0:1], es_p[:, 1:2])
                psq = pst()
                for k in range(NCH):
                    nc.tensor.transpose(psq[:, k * WV:(k + 1) * WV],
                                        e_sb[:, k * H:(k + 1) * H],
                                        ident[0:WV, 0:WV])
                nc.vector.tensor_copy(ew[:],
                                      psq[:, 0:NCH * WV].rearrange(
                                          "p (k j) -> p k j", k=NCH))

                # --- c softmax (on-chip transpose) ---
                ps_sc2 = pst()
                nc.tensor.transpose(ps_sc2[0:WV, 0:64], stt["sc_sb"][:],
                                    ident[0:64, 0:64])
                negmax = sm.tile([WV, 1], F32, name="negmax", tag="negmax")
                nc.vector.tensor_reduce(negmax[:], ps_sc2[0:WV, 0:64], AX.X,
                                        OP.max, negate=True)
                eac = sm.tile([WV, 64], F32, name="eac", tag="eac", bufs=1)
                sumec = sm.tile([WV, 1], F32, name="sumec", tag="sumec")
                nc.scalar.activation(eac[:], ps_sc2[0:WV, 0:64], A.Exp,
                                     bias=negmax[:], accum_out=sumec[:])
                sume2 = sm.tile([WV, 1], F32, name="sume2", tag="sume2",
                                bufs=1)
                nc.vector.tensor_scalar(sume2[:], sumec[:], 1e-6, None,
                                        OP.add)
                rec_c = sm.tile([WV, 1], F32, name="rec_c", tag="rec_c")
                nc.vector.reciprocal(rec_c[:], sume2[:])
                ac_nm = sm.tile([WV, 64], F32, name="ac_nm", tag="ac_nm",
                                bufs=1)
                nc.vector.tensor_scalar(ac_nm[:], eac[:], rec_c[:], None,
                                        OP.mult)
                ps_ac = pst()
                nc.tensor.transpose(ps_ac[0:64, 0:WV], ac_nm[:],
                                    ident[0:WV, 0:WV])
                ac_cm = sm.tile([64, WV], F32R, name="ac_cm", tag="ac_cm")
                nc.vector.tensor_copy(ac_cm[:], ps_ac[0:64, 0:WV])

                sm1 = pssm()
                # --- pf cols: stationary pe_nm chunk (bf16), moving e col ---
                for j in range(WV):
                    for k in range(NCH):
                        nc.tensor.matmul(sm1[:, PF0 + j:PF0 + j + 1],
                                         penm[j][:, k, :], ew[:, k, j:j + 1],
                                         start=(k == 0), stop=(k == NCH - 1))

                # --- cf cols + 1/esum broadcast ---
                for j in range(WV):
                    g = gs + LPERM[j]
                    nc.tensor.matmul(sm1[:, CF0 + j:CF0 + j + 1],
                                     ce_nm[:, g, :].bitcast(F32),
                                     ac_cm[:, j:j + 1].bitcast(F32),
                                     start=True, stop=True)
                es2 = sm.tile([WV, 1], F32, name="es2", tag="es2", bufs=1)
                nc.vector.tensor_scalar(es2[:], esum[:], 1e-6, None, OP.add)
                rec_p = sm.tile([WV, 1], F32, name="rec_p", tag="rec_p")
                nc.vector.reciprocal(rec_p[:], es2[:])
                ps_rt = pst()
                nc.tensor.transpose(ps_rt[0:1, 0:WV], rec_p[:],
                                    ident[0:WV, 0:WV])
                rec_row = sm.tile([1, WV], F32, name="rec_row", tag="rec_row")
                nc.vector.tensor_copy(rec_row[:], ps_rt[0:1, 0:WV])
                nc.tensor.matmul(sm1[:, BC0:BC0 + WV], onesr[:], rec_row[:],
                                 start=True, stop=True)

                # --- pfn / x ---
                rec_rep = sm.tile([H, WV], F32, name="rec_rep", tag="rec_rep")
                nc.vector.tensor_copy(rec_rep[:], sm1[:, BC0:BC0 + WV])
                pfnT = sm.tile([H, WV], F32R, name="pfnT", tag="pfnT")
                nc.vector.tensor_mul(pfnT[:], sm1[:, PF0:PF0 + WV],
                                     rec_rep[:])
                if i == D - 1:
                    cf_sb = sm.tile([H, WV], F32, name="cf_sb", tag="cf_sb",
                                    bufs=1)
                    nc.vector.tensor_copy(cf_sb[:], sm1[:, CF0:CF0 + WV])
                    ps_hx = pst()
                    nc.tensor.transpose(ps_hx[0:WV, 0:H], cf_sb[:], ident[:])
                    nc.vector.tensor_copy(xcf[:], ps_hx[0:WV, 0:H])
                    ps_hx2 = pst()
                    nc.tensor.transpose(ps_hx2[0:WV, 0:H], pfnT[:].bitcast(F32),
                                        ident[:])
                    nc.vector.tensor_copy(pfn_nm[:], ps_hx2[0:WV, 0:H])
                    continue  # GRU output unused after the last iteration

                xT = sm.tile([H, WV], F32R, name="xT", tag="xT")
                nc.vector.tensor_mul(xT[:], sm1[:, CF0:CF0 + WV],
                                     pfnT[:].bitcast(F32))

                # --- GRU ---
                nc.tensor.matmul(sm1[0:WV, GI0:GI0 + 384], xT[:],
                                 wp[:, W_IH:W_IH + 384], start=True, stop=True)
                ps_gh = pst()
                nc.tensor.matmul(ps_gh[0:WV, 0:384], mT[:],
                                 wp[:, W_HH:W_HH + 384], start=True, stop=True)
                # deferred second lookahead front: queued PE work that runs
                # while the GRU DVE/Act chain and mcp(i+1) latency resolve
                front(i + 1, 1)
                gi = sm.tile([WV, 384], F32, name="gi", tag="gi", bufs=1)
                nc.vector.tensor_add(gi[:], sm1[0:WV, GI0:GI0 + 384],
                                     bg[:, BG_IH:BG_IH + 384])
                gh = sm.tile([WV, 384], F32, name="gh", tag="gh", bufs=1)
                nc.vector.tensor_add(gh[:], ps_gh[0:WV, 0:384],
                                     bg[:, BG_HH:BG_HH + 384])
                rz_pre = sm.tile([WV, 256], F32, name="rz_pre", tag="t256",
                                 bufs=1)
                nc.vector.tensor_add(rz_pre[:], gi[:, 0:256], gh[:, 0:256])
                rz_t = sm.tile([WV, 256], F32, name="rz_t", tag="rz_t",
                               bufs=1)
                nc.scalar.activation(rz_t[:], rz_pre[:], A.Tanh, scale=0.5)
                rz = sm.tile([WV, 256], F32, name="rz", tag="rz", bufs=1)
                nc.vector.tensor_scalar(rz[:], rz_t[:], 0.5, 0.5, OP.mult,
                                        OP.add)
                n_pre = sm.tile([WV, H], F32, name="n_pre", tag="n_pre",
                                bufs=1)
                nc.vector.tensor_mul(n_pre[:], rz[:, 0:H], gh[:, 256:384])
                n_pre2 = sm.tile([WV, H], F32, name="n_pre2", tag="n_pre2",
                                 bufs=1)
                nc.vector.tensor_add(n_pre2[:], n_pre[:], gi[:, 256:384])
                n_t = sm.tile([WV, H], F32, name="n_t", tag="n_t", bufs=1)
                nc.scalar.activation(n_t[:], n_pre2[:], A.Tanh)
                dmn = sm.tile([WV, H], F32, name="dmn", tag="dmn", bufs=1)
                nc.vector.tensor_sub(dmn[:], m_nm[:], n_t[:])
                zd = sm.tile([WV, H], F32, name="zd", tag="zd", bufs=1)
                nc.vector.tensor_mul(zd[:], rz[:, H:256], dmn[:])
                m_nm = sm.tile([WV, H], F32, name="m_nm", tag="m_nm", bufs=1)
                nc.vector.tensor_add(m_nm[:], n_t[:], zd[:])
                ps_mT = pst()
                nc.tensor.transpose(ps_mT[0:H, 0:WV], m_nm[:],
                                    ident[0:WV, 0:WV])
                mT = sm.tile([H, WV], F32R, name="mT", tag="mT")
                nc.vector.tensor_copy(mT[:], ps_mT[0:H, 0:WV])
                mcp_chain(i + 1, mT)

            # head input rows, ready early so the end-of-kernel head chains
            # can overlap the other wave's tail (DVE only; the head DMAs and
            # matmuls stay at the end of the program)
            acf = sm.tile([WV, 2 * H], F32R, name="acf", tag=f"acf{w}",
                          bufs=1)
            nc.vector.tensor_copy(acf[:, 0:H], xcf[:])
            nc.vector.tensor_copy(acf[:, H:2 * H], sf_w[:].bitcast(F32))
            acf_w.append(acf)

        # ----- heads (after both waves so they fill engine-queue gaps) -----
        for w in range(NW):
            gs = w * WV
            acf = acf_w[w]
            for j in range(WV):
                q = gs + j
                pfr = st.tile([1, H], F32R, name="pfr", tag="pfr")
                nc.sync.dma_start(out=pfr[:], in_=pfn_w[w][j:j + 1, :])
                acfr = st.tile([1, 2 * H], F32R, name="acfr", tag="acfr")
                nc.sync.dma_start(out=acfr[:], in_=acf[j:j + 1, :])
                ps_o = psb()
                nc.tensor.matmul(ps_o[:, 0:256], pfr[:], acfr[:],
                                 start=True, stop=True)
                gk = st.tile([H, 2 * H], F32, name="gk", tag="gk", bufs=1)
                nc.scalar.activation(gk[:], ps_o[:, 0:256], A.Prelu,
                                     alpha=0.1)
                gkw = st.tile([H, 2 * H], F32, name="gkw", tag="gkw", bufs=1)
                nc.vector.scalar_tensor_tensor(gkw[:], gk[:], 1.0, w2t[:],
                                               OP.mult, OP.mult,
                                               accum_out=partials[:, q:q + 1])

        # ---------- output ----------
        ps_fin = pssm()
        nc.tensor.matmul(ps_fin[0:G, FIN0:FIN0 + 1], partials[:].bitcast(F32),
                         bp[:, B_ONES:B_ONES + 1],
                         start=True, stop=True)
        ofin = gl.tile([G, 1], F32, name="ofin", tag="ofin")
        nc.vector.tensor_scalar(ofin[:], ps_fin[0:G, FIN0:FIN0 + 1],
                                float(b_out_val), None, OP.add)
        nc.sync.dma_start(out=out_d[:], in_=ofin[:])

    if split:
        _split_waits(nc)
    return nc


def kernel(**inputs) -> np.ndarray:
    f = {k: np.asarray(v) for k, v in inputs.items()}
    f = {k: (v.astype(np.float32) if v.dtype != np.int32 else v)
         for k, v in f.items()}

    wcols = np.zeros((H, WCOLS), dtype=np.float32)
    wcols[:, W_PC:W_PC + H] = f['W_pc']
    wcols[:, W_CAFF:W_CAFF + H] = f['W_caff']
    wcols[:, W_PP:W_PP + H] = f['W_pp']
    wcols[:, W_PAFF:W_PAFF + H] = f['W_paff']
    for i in range(D):
        wcols[:, W_P2C(i):W_P2C(i) + H] = f['W_p2c'][i]
        wcols[:, W_HP0(i):W_HP0(i) + H] = f['W_hp0'][i]
        wcols[:, W_HC0(i):W_HC0(i) + H] = f['W_hc0'][i]
        wcols[:, W_C2P(i):W_C2P(i) + H] = f['W_c2p'][i]
        wcols[:, W_MCP(i):W_MCP(i) + 256] = np.concatenate(
            [f['W_mc1'][i], f['W_mp1'][i]], axis=1)
        wcols[:, W_HC1(i):W_HC1(i) + 1] = f['W_hc1'][i]
        wcols[:, W_HP1(i):W_HP1(i) + 1] = f['W_hp1'][i]
    wcols[:, W_IH:W_IH + 384] = f['W_ih']
    wcols[:, W_HH:W_HH + 384] = f['W_hh']
    wcols[:, W_SAFF:W_SAFF + H] = f['W_saff']
    wcols[:, W_ONES] = 1.0

    bpk = np.zeros((H, BCOLS), dtype=np.float32)
    bpk[:, B_PP] = f['b_pp']
    bpk[:, B_PAFF] = f['b_paff']
    bpk[:, B_PC] = f['b_pc']
    bpk[:, B_CAFF] = f['b_caff']
    bpk[:, B_ONES] = 1.0
    for i in range(D):
        bpk[:, B_P2C(i)] = f['b_p2c'][i]
        bpk[:, B_HP0(i)] = f['b_hp0'][i]
        bpk[:, B_HC0(i)] = f['b_hc0'][i]
        bpk[:, B_C2P(i)] = f['b_c2p'][i]
        bpk[:, B_HP1(i)] = float(f['b_hp1'][i][0])

    bgw = np.zeros((WV, BGCOLS), dtype=np.float32)
    bgw[:, BG_SAFF:BG_SAFF + H] = np.tile(f['b_saff'][None, :], (WV, 1))
    for i in range(D):
        bgw[:, BG_MCP(i):BG_MCP(i) + 256] = np.tile(
            np.concatenate([f['b_mc1'][i], f['b_mp1'][i]])[None, :], (WV, 1))
    bgw[:, BG_IH:BG_IH + 384] = np.tile(f['b_ih'][None, :], (WV, 1))
    bgw[:, BG_HH:BG_HH + 384] = np.tile(f['b_hh'][None, :], (WV, 1))

    w2t = np.ascontiguousarray(f['W_out'].reshape(2 * H, H).T, dtype=np.float32)
    b_out_val = float(f['b_out'][0])
    b_hp1 = [float(f['b_hp1'][i][0]) for i in range(D)]

    key = ('nc3', b_out_val, tuple(b_hp1))
    if key not in _CACHE:
        _CACHE[key] = _build(b_out_val, b_hp1)
    nc = _CACHE[key]

    comp = f['comp_feature'].reshape(NCORES, G, NC, H)
    prot = f['prot_feature'].reshape(NCORES, G, NP, H)
    gomp = f['gomp_feature'].reshape(NCORES, G, H)
    in_maps = []
    for c in range(NCORES):
        in_maps.append({
            "protT": np.ascontiguousarray(
                prot[c].transpose(2, 0, 1).reshape(H, G * NP)),
            "compT": np.ascontiguousarray(
                comp[c].transpose(2, 0, 1).reshape(H, G * NC)),
            "gompT": np.ascontiguousarray(gomp[c].T[:, GPERM]),
            "wpack": wcols, "bpack": bpk, "bgw": bgw, "w2t": w2t,
        })

    global LAST_EXEC_NS, LAST_RESULT
    try:
        r = run_bass_kernel_spmd(nc, in_maps, list(range(NCORES)), trace=TRACE)
    except ModuleNotFoundError:
        r = run_bass_kernel_spmd(nc, in_maps, list(range(NCORES)))
    LAST_RESULT = r
    if getattr(r, "exec_time_ns", None):
        LAST_EXEC_NS = r.exec_time_ns
    res = r.results
    inv = np.empty(G, dtype=np.int64)
    inv[GPERM] = np.arange(G)
    return np.concatenate([res[c]["out"][inv] for c in range(NCORES)], axis=0)
